# revision 42
# baseline (speedup 1.0000x reference)
"""DenseCrossEntropyLoss kernel for 8 Trainium2 NeuronCores.

Data-parallel over the batch dim (B=32 -> 4 batches/core). Each core:
  - l2-normalizes its gathered query vectors,
  - computes query x target-grid logits (K=256 contraction, fp32r matmuls),
  - normalizes logits by per-cell target norms (ones-matmul broadcast trick),
  - computes a fixed-shift logsumexp over the 4096 grid cells,
  - emits masked per-keypoint partial terms.
Host gathers the per-core partials and finishes the masked mean.
"""

import sys

for _p in ("/opt/trn_rl_repo", "/root/.axon_site/_ro/trn_rl_repo"):
    if _p not in sys.path:
        sys.path.insert(0, _p)

import numpy as np

import concourse.bacc as bacc
import concourse.tile as tile
from concourse import mybir
from concourse.bass_utils import run_bass_kernel_spmd

F32 = mybir.dt.float32
F32R = mybir.dt.float32r
AF = mybir.ActivationFunctionType

# Problem shapes (hardcoded per spec).
B, C, HF, WF, N = 32, 256, 64, 64, 64
M = HF * WF          # 4096 target cells
STRIDE = 16
TEMPERATURE = 0.01
N_CORES = 8
BPC = B // N_CORES   # batches per core = 4
KT = C // 128        # k-tiles = 2
LN_INV_T = float(np.log(1.0 / TEMPERATURE))

MCHUNK = 1024        # m-chunk (free dim) per pipeline step
NCHUNK = M // MCHUNK

# engine assignment for the per-chunk square / f32r-rounding passes, per k-tile
SQ_ENGINES = ("scalar", "vector")
RND_ENGINES = ("gpsimd", "vector")

# Fixed logsumexp shift. |logit| <= 1/T = 100 (Cauchy-Schwarz on unit
# vectors), so exp(logit - 15) <= e^85 < f32 max; HW exp flushes args
# below -87 to 0, which is harmless in the sum. Added back on the host.
SHIFT = 15.0
# HW Ln is only valid on [2^-66, 2^64]; se reaches ~3e29, so feed the
# final Ln as ln(se * 2^-LN_SCALE_BITS) and add LN_SCALE_BITS*ln2 back
# on the host.
LN_SCALE_BITS = 44

ACT_SET = "natural_log_exp_and_others"


def _pin_act_table(nc):
    """Force every ACT function to resolve to ACT_SET (which contains
    square/ln/exp) so bacc emits exactly one table load instead of
    thrashing between per-function favorite sets (~2.7us per load)."""
    import concourse.hw_specs as hw_specs

    tables = hw_specs.get_activation_tables(nc.m.arch)  # functools.cache'd
    assert ACT_SET in tables
    for k in list(tables.keys()):
        if k != ACT_SET:
            tables[k] = set()  # in-place: preserves set-id indices


def _engine(nc, name):
    return {"scalar": nc.scalar, "vector": nc.vector, "gpsimd": nc.gpsimd}[name]


def build_bass():
    nc = bacc.Bacc("TRN2", target_bir_lowering=False, debug=False)
    _pin_act_table(nc)

    ft_d = nc.dram_tensor("ft", [BPC, KT, 128, M], F32, kind="ExternalInput")
    q_d = nc.dram_tensor("q", [BPC, KT, 128, N], F32, kind="ExternalInput")
    ftt_d = nc.dram_tensor("ftt", [BPC, KT, 128, N], F32, kind="ExternalInput")
    mask_pairs_d = nc.dram_tensor("mask_pairs", [128, BPC // 2], F32,
                                  kind="ExternalInput")
    mask_rows_d = nc.dram_tensor("mask_rows", [1, BPC * N], F32,
                                 kind="ExternalInput")
    # out1[pair, p] = mask * logsumexp(logits) for keypoint row p
    out1_d = nc.dram_tensor("out1", [BPC // 2, 128, 1], F32, kind="ExternalOutput")
    # out2[b*N + n] = mask * tlogit
    out2_d = nc.dram_tensor("out2", [1, BPC * N], F32, kind="ExternalOutput")

    with tile.TileContext(nc) as tc:
        with (
            tc.tile_pool(name="consts", bufs=1) as consts,
            tc.tile_pool(name="tail", bufs=1) as tail,
            tc.tile_pool(name="ftp", bufs=3) as ftp,
            tc.tile_pool(name="sqp", bufs=2) as sqp,
            tc.tile_pool(name="work", bufs=2) as work,
            tc.tile_pool(name="stats", bufs=2) as stats,
            tc.tile_pool(name="psum", bufs=2, space="PSUM") as psum,
        ):
            # ---- constants ----
            ones_f = consts.tile([128, 128], F32)
            nc.vector.memset(ones_f, 1.0)
            ones128 = consts.tile([128, 128], F32R)
            nc.vector.tensor_copy(out=ones128, in_=ones_f)
            biasln = consts.tile([128, 1], F32)
            nc.vector.memset(biasln, LN_INV_T)
            biasshift = consts.tile([128, 1], F32)
            nc.vector.memset(biasshift, -SHIFT)
            # [ones|0] / [0|ones] padded stationaries (batch half-select);
            # fp32r matmuls must write PSUM partition base 0, so batch packing
            # is done via zero-padded stationary columns + PSUM accumulation.
            # NB: all memset regions are disjoint — overlapping writers on one
            # tile race under Tile scheduling.
            opad_f = consts.tile([128, 2, 128], F32)
            nc.vector.memset(opad_f[:, 0, 0:64], 1.0)
            nc.vector.memset(opad_f[:, 0, 64:128], 0.0)
            nc.vector.memset(opad_f[:, 1, 0:64], 0.0)
            nc.vector.memset(opad_f[:, 1, 64:128], 1.0)
            opad = consts.tile([128, 2, 128], F32R)
            nc.vector.tensor_copy(out=opad, in_=opad_f)
            maskp = consts.tile([128, BPC // 2], F32)
            nc.sync.dma_start(out=maskp, in_=mask_pairs_d[:, :])
            maskr = consts.tile([1, BPC * N], F32)
            nc.sync.dma_start(out=maskr, in_=mask_rows_d[:, :])

            # ---- tail part A: query normalization (all batches at once) ----
            # q_sb[p, b, k, n]
            q_sb = tail.tile([128, BPC, KT, N], F32)
            nc.sync.dma_start(
                out=q_sb,
                in_=q_d[:, :, :, :].rearrange("b k p n -> p b k n"))
            ftt_sb = tail.tile([128, BPC, KT, N], F32)
            nc.sync.dma_start(
                out=ftt_sb,
                in_=ftt_d[:, :, :, :].rearrange("b k p n -> p b k n"))

            sqq = tail.tile([128, BPC, KT, N], F32R)
            nc.scalar.activation(out=sqq, in_=q_sb, func=AF.Square)
            # Pq[p, b, n] = ssq of query (b, n), broadcast down partitions
            pq = psum.tile([128, BPC, N], F32, tag="R")
            for k in range(KT):
                nc.tensor.matmul(pq, ones128[:, :], sqq[:, :, k, :],
                                 start=(k == 0), stop=(k == KT - 1))
            uq = tail.tile([128, BPC, N], F32)
            nc.scalar.activation(out=uq, in_=pq, func=AF.Ln)
            rq = tail.tile([128, BPC, N], F32)  # 1/||q||
            nc.scalar.activation(out=rq, in_=uq, func=AF.Exp, scale=-0.5)
            # zero-padded stationaries: batch bi of a pair occupies stationary
            # columns [64*bi, 64*bi+64); the other half is zero.
            qstage = tail.tile([128, BPC, KT, 128], F32)
            for k in range(KT):
                for b in range(BPC):
                    h = (b % 2) * 64
                    oh = 64 - h
                    nc.vector.memset(qstage[:, b, k, oh:oh + 64], 0.0)
                    nc.vector.tensor_mul(qstage[:, b, k, h:h + 64],
                                         q_sb[:, b, k, :], rq[:, b, :])
            qpad = tail.tile([128, BPC, KT, 128], F32R)
            nc.vector.tensor_copy(out=qpad, in_=qstage)

            # ---- tail part B: target logit (gathered target columns) ----
            sqt = tail.tile([128, BPC, KT, N], F32R)
            nc.scalar.activation(out=sqt, in_=ftt_sb, func=AF.Square)
            z = tail.tile([128, BPC, KT, N], F32R)
            for k in range(KT):
                for b in range(BPC):
                    h = (b % 2) * 64
                    nc.vector.tensor_mul(z[:, b, k, :],
                                         qstage[:, b, k, h:h + 64],
                                         ftt_sb[:, b, k, :])
            pt = psum.tile([1, BPC, N], F32, tag="R")
            pz = psum.tile([1, BPC, N], F32, tag="S")
            for k in range(KT):
                nc.tensor.matmul(pt, ones128[:, 0:1], sqt[:, :, k, :],
                                 start=(k == 0), stop=(k == KT - 1))
            for k in range(KT):
                nc.tensor.matmul(pz, ones128[:, 0:1], z[:, :, k, :],
                                 start=(k == 0), stop=(k == KT - 1))
            ut = tail.tile([1, BPC, N], F32)
            nc.scalar.activation(out=ut, in_=pt, func=AF.Ln)
            rt = tail.tile([1, BPC, N], F32)  # (1/T) / ||ft_t||
            nc.scalar.activation(out=rt, in_=ut, func=AF.Exp, scale=-0.5,
                                 bias=biasln[0:1, :])
            tl = tail.tile([1, BPC, N], F32)
            nc.vector.tensor_mul(tl, pz, rt)
            z2 = tail.tile([1, BPC, N], F32)
            nc.vector.tensor_mul(
                z2, tl, maskr[:, :].rearrange("o (b n) -> o b n", n=N))
            nc.sync.dma_start(out=out2_d[:, :],
                              in_=z2[:, :, :].rearrange("p b n -> p (b n)"))

            # ---- main loop: pairs of batches x m-chunks ----
            npairs = BPC // 2
            for pair in range(npairs):
                b0, b1 = 2 * pair, 2 * pair + 1
                # Split the globally-last chunk in two: shortens the
                # compute-drain chain after the final DMA lands.
                chunks = [(i * MCHUNK, MCHUNK) for i in range(NCHUNK)]
                if pair == npairs - 1:
                    ms0 = (NCHUNK - 1) * MCHUNK
                    h = MCHUNK // 2
                    chunks = chunks[:-1] + [(ms0, h), (ms0 + h, h)]
                separt = stats.tile([128, len(chunks)], F32, tag="separt")
                for mc, (ms, mlen) in enumerate(chunks):
                    # coalesced DMAs: one per k-tile, both batches
                    ft_t = ftp.tile([128, KT, 2, mlen], F32, tag="ft")
                    for k in range(KT):
                        nc.sync.dma_start(
                            out=ft_t[:, k, :, :],
                            in_=ft_d[b0:b0 + 2, k, :, ms:ms + mlen]
                            .rearrange("b p m -> p b m"))
                    frtiles = {}
                    sqtiles = {}
                    for k in range(KT):
                        t = ft_t[:, k, :, :]
                        # round to f32r for the PE (both batches in one op)
                        fr = ftp.tile([128, 2, mlen], F32R, tag=f"fr{k}")
                        _engine(nc, RND_ENGINES[k]).tensor_copy(out=fr, in_=t)
                        frtiles[k] = fr
                        sq = sqp.tile([128, 2, mlen], F32R, tag=f"sq{k}")
                        if SQ_ENGINES[k] == "scalar":
                            nc.scalar.activation(out=sq, in_=t, func=AF.Square)
                        else:
                            _engine(nc, SQ_ENGINES[k]).tensor_mul(sq, t, t)
                        sqtiles[k] = sq

                    rps = psum.tile([128, mlen], F32, tag="R")
                    sps = psum.tile([128, mlen], F32, tag="S")
                    for s in range(mlen // 512):
                        sl = slice(512 * s, 512 * (s + 1))
                        for i, (bi, k) in enumerate(
                                [(b, k) for b in range(2) for k in range(KT)]):
                            nc.tensor.matmul(
                                rps[:, sl],
                                qpad[:, 2 * pair + bi, k, :],
                                frtiles[k][:, bi, sl],
                                start=(i == 0), stop=(i == 2 * KT - 1))
                        for i, (bi, k) in enumerate(
                                [(b, k) for b in range(2) for k in range(KT)]):
                            nc.tensor.matmul(
                                sps[:, sl],
                                opad[:, bi, :],
                                sqtiles[k][:, bi, sl],
                                start=(i == 0), stop=(i == 2 * KT - 1))

                    u = work.tile([128, mlen], F32, tag="u")
                    nc.scalar.activation(out=u, in_=sps, func=AF.Ln)
                    st = work.tile([128, mlen], F32, tag="st")
                    nc.scalar.activation(out=st, in_=u, func=AF.Exp, scale=-0.5,
                                         bias=biasln)
                    logits = work.tile([128, mlen], F32, tag="logits")
                    nc.vector.tensor_mul(logits, rps, st)
                    # e is scratch: overwrite the dead rps PSUM bank
                    # (ScE->PSUM is the faster ACT port).
                    nc.scalar.activation(out=rps, in_=logits, func=AF.Exp,
                                         bias=biasshift,
                                         accum_out=separt[:, mc:mc + 1])

                se = stats.tile([128, 1], F32, tag="se")
                nc.vector.reduce_sum(se, separt, axis=mybir.AxisListType.X)
                lnse = stats.tile([128, 1], F32, tag="lnse")
                nc.scalar.activation(out=lnse, in_=se, func=AF.Ln,
                                     scale=float(2.0 ** -LN_SCALE_BITS))
                z1 = stats.tile([128, 1], F32, tag="z1")
                nc.vector.tensor_mul(z1, lnse, maskp[:, pair:pair + 1])
                nc.sync.dma_start(out=out1_d[pair], in_=z1[:, :])

    nc.compile()
    return nc


_NC_CACHE = None
LAST_RESULTS = None


def _get_nc():
    global _NC_CACHE
    if _NC_CACHE is None:
        _NC_CACHE = build_bass()
    return _NC_CACHE


def kernel(feats_src, feats_trg, kps_src, kps_trg, kps_mask):
    feats_src = np.ascontiguousarray(np.asarray(feats_src, dtype=np.float32))
    feats_trg = np.ascontiguousarray(np.asarray(feats_trg, dtype=np.float32))
    kps_src = np.asarray(kps_src, dtype=np.float32)
    kps_trg = np.asarray(kps_trg, dtype=np.float32)
    mask_f = np.asarray(kps_mask).astype(np.float32)

    fs = feats_src.reshape(B, C, M)
    ft = feats_trg.reshape(B, C, M)

    def flat_idx(kps):
        fx = np.clip((kps[..., 0] / STRIDE).astype(np.int32), 0, WF - 1)
        fy = np.clip((kps[..., 1] / STRIDE).astype(np.int32), 0, HF - 1)
        return fy * WF + fx  # (B, N)

    src_idx = flat_idx(kps_src)
    trg_idx = flat_idx(kps_trg)

    # Gather raw query/target columns (sharding prep; normalization happens
    # on device).
    q_raw = np.take_along_axis(fs, src_idx[:, None, :], axis=2)   # (B, C, N)
    ftt_raw = np.take_along_axis(ft, trg_idx[:, None, :], axis=2)  # (B, C, N)

    nc = _get_nc()
    in_maps = []
    for c in range(N_CORES):
        sl = slice(BPC * c, BPC * (c + 1))
        m = mask_f[sl]                                   # (BPC, N)
        mask_pairs = np.ascontiguousarray(
            m.reshape(BPC // 2, 2, N).transpose(1, 2, 0).reshape(128, BPC // 2))
        in_maps.append({
            "ft": np.ascontiguousarray(ft[sl].reshape(BPC, KT, 128, M)),
            "q": np.ascontiguousarray(q_raw[sl].reshape(BPC, KT, 128, N)),
            "ftt": np.ascontiguousarray(ftt_raw[sl].reshape(BPC, KT, 128, N)),
            "mask_pairs": mask_pairs,
            "mask_rows": np.ascontiguousarray(m.reshape(1, BPC * N)),
        })

    res = run_bass_kernel_spmd(nc, in_maps, core_ids=list(range(N_CORES)))
    global LAST_RESULTS
    LAST_RESULTS = res

    total = 0.0
    for r in res.results:
        total += float(r["out1"].astype(np.float64).sum())
        total -= float(r["out2"].astype(np.float64).sum())
    count = float(mask_f.sum())
    if count < 0.5:
        return np.float32(0.0)
    return np.float32(total / count + SHIFT + LN_SCALE_BITS * float(np.log(2.0)))
